# revision 70
# baseline (speedup 1.0000x reference)
"""Trainium2 Bass kernel for a causal multi-head attention block
(fused QKV proj + RoPE + causal softmax attention + out proj).

Sharding: 8 cores = 4 batches x 2 head-groups (8 heads each), no
on-chip collectives: each core emits a partial out-projection [N, C]
(row-parallel over heads); the host sums each batch's pair of partials
and adds the output bias.

Schedule (single fused stream, no phase barriers on the PE):
  - Prefix: chunk-major projection of q0/k0 (pair 0) across 6 PSUM
    banks tracking the wt/xt chunk DMA stream, then v tiles 0-3.
  - Attention runs in two passes over head-pairs (hp0, hp1 across all
    q-blocks, then hp2/hp3 interleaved per block). The score -> exp ->
    PV software pipeline is carried ACROSS block boundaries; each
    iteration drains one item from a fill queue (v4-15 projections,
    later q/k pairs + per-block RoPE, deferred oT transposes, split
    out-proj chains) BEFORE the PV flush, so fill absorbs the exp
    wait instead of the PE head-of-line stalling on it.
  - Scores S^T[k, q] for both heads row-packed into one [128,1024]
    PSUM tile, causal-trimmed on diagonal tiles; the causal mask is
    applied ON THE PE (identity-lhsT matmul accumulating a 0/-1e9
    table) so DVE never sits in the exp dependency chain.
  - PV runs in natural layout: per q-tile [128,65] outputs (v plus a
    ones column giving the softmax denominator Z), so normalization
    is per-partition: DVE recip of the strided Z columns + one
    broadcast multiply per two heads; a cheap PE transpose (deferred
    as fill) rebuilds oT bf16 for the out-projection.
  - Out-proj (bf16 operands) per q-block is enqueued as fill one
    block behind its norm; the first and last blocks split their
    head-group accumulation so the pass-2 lead-in and the kernel tail
    stay short.
Scores/PV/projections in bf16 (f32 PSUM accumulation); v-bias via a
K=1 ones-matmul; q pre-scaled by D^-0.5 on the host; rel err vs the
fp32 reference ~7.4e-3.
"""

import sys

sys.path.insert(0, "/opt/trn_rl_repo")

import numpy as np

import concourse.bass as bass
import concourse.mybir as mybir
from concourse import bacc, library_config
from concourse.tile import TileContext

F32 = mybir.dt.float32
F32R = mybir.dt.float32r
BF16 = mybir.dt.bfloat16

B, N, C = 4, 2048, 1024
H_ALL, D = 16, 64
HPC = 8  # heads per core
JQK = HPC * D  # 512 rows for q (and k) per core
ROPE_THETA = 10000.0
SCALE = D**-0.5
NEG = -1e9

NT = N // 128  # 16 n-tiles
NB = N // 512  # 4 n-blocks
CC = C // 128  # 8 contraction chunks


def r(ap):
    return ap.bitcast(F32R)


# --- optional emission-order instrumentation (used by analyze.py only) ---
INSTRUMENT = False
PE_LABELS = []
_CUR = ["?"]


def _lbl(s):
    _CUR[0] = s


def build_nc(reps=1):
    PE_LABELS.clear()
    nc = bacc.Bacc(None, target_bir_lowering=False)
    if INSTRUMENT:
        _orig_mm = nc.tensor.matmul

        def _mm(*a, **k):
            PE_LABELS.append(_CUR[0])
            return _orig_mm(*a, **k)

        nc.tensor.matmul = _mm

    xt = nc.declare_dram_parameter("xt", [CC, 128, N], BF16, isOutput=False)
    wt = nc.declare_dram_parameter("wt", [CC, 128, 1536], BF16, isOutput=False)
    bqk = nc.declare_dram_parameter("bqk", [128, 8], F32, isOutput=False)
    bv = nc.declare_dram_parameter("bv", [1, JQK], F32R, isOutput=False)
    cosb = nc.declare_dram_parameter("cosb", [128, N], BF16, isOutput=False)
    sinb = nc.declare_dram_parameter("sinb", [128, N], BF16, isOutput=False)
    maskb = nc.declare_dram_parameter("maskb", [128, 128], BF16, isOutput=False)
    identb = nc.declare_dram_parameter("identb", [128, 128], BF16, isOutput=False)
    owt = nc.declare_dram_parameter("owt", [4, 128, C], BF16, isOutput=False)
    onesp = nc.declare_dram_parameter("onesp", [1, 128], F32R, isOutput=False)
    ones16 = nc.declare_dram_parameter("ones16", [128, 8], BF16, isOutput=False)
    out = nc.declare_dram_parameter("out", [N, C], F32, isOutput=True)

    with TileContext(nc) as tc:
      for _rep in range(reps):
        with tc.tile_pool(name="persist", bufs=1) as pp:
            qkT = [pp.tile([128, N], BF16, tag=f"qkT{t}", name=f"qkT{t}") for t in range(8)]
            vN = [pp.tile([128, HPC * 65], BF16, tag=f"vN{t}", name=f"vN{t}") for t in range(NT)]
            oT = [pp.tile([128, N], BF16, tag=f"oT{t}", name=f"oT{t}") for t in range(4)]
            owt_sb = [pp.tile([128, C], BF16, tag=f"owt{hc}", name=f"owt{hc}") for hc in range(4)]
            cos_sb = pp.tile([128, N], BF16, tag="cos_sb", name="cos_sb")
            sin_sb = pp.tile([128, N], BF16, tag="sin_sb", name="sin_sb")
            mask_sb = pp.tile([128, 128], BF16, tag="mask_sb", name="mask_sb")
            ident_sb = pp.tile([128, 128], BF16, tag="ident_sb", name="ident_sb")
            bqk_sb = pp.tile([128, 8], F32, tag="bqk_sb", name="bqk_sb")
            bv_sb = pp.tile([1, JQK], F32R, tag="bv_sb", name="bv_sb")
            ones_sb = pp.tile([1, 128], F32R, tag="ones_sb", name="ones_sb")
            ones16_sb = pp.tile([128, 8], BF16, tag="ones16_sb", name="ones16_sb")
            xts = [pp.tile([128, N], BF16, tag=f"xt{cch}", name=f"xt{cch}") for cch in range(CC)]
            wt_sb = [pp.tile([128, 1536], BF16, tag=f"wt{cch}", name=f"wt{cch}") for cch in range(CC)]

            # input DMAs in consumption order: per chunk wt then two halves
            # of xt (half pieces advance the chunk-major prefix earlier
            # without blowing the serial HWDGE desc-gen budget); tables
            # after the chunks; owt last (first consumed ~80us in).
            for cch in range(CC):
                nc.sync.dma_start(out=wt_sb[cch][:, :], in_=wt[cch, :, :])
                for nbp in range(2):
                    nc.sync.dma_start(
                        out=xts[cch][:, nbp * 1024 : (nbp + 1) * 1024],
                        in_=xt[cch, :, nbp * 1024 : (nbp + 1) * 1024],
                    )
                if cch == 1:
                    nc.sync.dma_start(out=bqk_sb[:, :], in_=bqk[:, :])
                    nc.sync.dma_start(out=ones16_sb[:, :], in_=ones16[:, :])
                    nc.sync.dma_start(out=bv_sb[:, :], in_=bv[:, :])
                    nc.sync.dma_start(out=ones_sb[:, :], in_=onesp[:, :])
            nc.sync.dma_start(out=cos_sb[:, :], in_=cosb[:, :])
            nc.sync.dma_start(out=sin_sb[:, :], in_=sinb[:, :])
            nc.sync.dma_start(out=mask_sb[:, :], in_=maskb[:, :])
            nc.sync.dma_start(out=ident_sb[:, :], in_=identb[:, :])
            for hc in range(4):
                nc.sync.dma_start(out=owt_sb[hc][:, :], in_=owt[hc, :, :])

            # gpsimd: library + the ones column (col 64 of each head group)
            # for every v tile - independent of the v projections
            nc.gpsimd.load_library(library_config.attn)
            for t in range(NT):
                nc.gpsimd.tensor_copy(
                    out=vN[t][:, 64 : HPC * 65 : 65], in_=ones16_sb[:, :]
                )

            with tc.tile_pool(name="rope", bufs=2) as rp:
                sw_cache = {}

                def get_sw(jt):
                    # one sw tile per jt, shared by its rope block-items;
                    # 2 rotating buffers (jt usage windows are sequential)
                    if jt not in sw_cache:
                        sw_cache[jt] = rp.tile(
                            [128, N], BF16, tag="swf", name=f"swf{jt}", bufs=3
                        )
                    return sw_cache[jt]

                def rope_block(jt, nbp):
                    # r[a] = q[a]*cos[a] + q[a^1]*sinSigned[a] per 32-block,
                    # applied to one 512-wide n-block so the first consumer
                    # never waits on a full-row DVE chain
                    sw = get_sw(jt)
                    s = slice(nbp * 512, (nbp + 1) * 512)
                    for a in range(4):
                        b = (a ^ 1) * 32
                        nc.vector.tensor_mul(
                            sw[a * 32 : a * 32 + 32, s],
                            qkT[jt][b : b + 32, s],
                            sin_sb[b : b + 32, s],
                        )
                    nc.vector.tensor_mul(qkT[jt][:, s], qkT[jt][:, s], cos_sb[:, s])
                    nc.vector.tensor_add(qkT[jt][:, s], qkT[jt][:, s], sw[:, s])

                # ---- prefix: pair 0 (q=jt0, k=jt4) chunk-major across 8
                # PSUM banks so PE tracks the chunk DMA stream ----
                with tc.tile_pool(name="prefix_ps", bufs=1, space="PSUM") as pfx:
                    # 6 banks only: the other two stay untouched so the
                    # fill pool's first chains never wait on prefix evacs
                    pf = {
                        (jt, nbp): pfx.tile(
                            [128, 512], F32, tag=f"pf{jt}_{nbp}", name=f"pf{jt}_{nbp}"
                        )
                        for jt in (0, 4)
                        for nbp in range(3)
                    }
                    _lbl("prefix")
                    for cch in range(CC):
                        for nbp in range(3):
                            for jt in (0, 4):
                                nc.tensor.matmul(
                                    pf[(jt, nbp)][:, :],
                                    wt_sb[cch][:, jt * 128 : (jt + 1) * 128],
                                    xts[cch][:, nbp * 512 : (nbp + 1) * 512],
                                    start=(cch == 0),
                                    stop=(cch == CC - 1),
                                )
                    # all evacs first (each frees a PSUM bank; keeps the DVE
                    # queue short ahead of the v evacuations), then only the
                    # nb0 rope blocks -- the rest run after v0-3 below
                    for nbp in range(3):
                        for jt in (0, 4):
                            nc.scalar.activation(
                                qkT[jt][:, nbp * 512 : (nbp + 1) * 512],
                                pf[(jt, nbp)][:, :],
                                mybir.ActivationFunctionType.Identity,
                                bias=bqk_sb[:, jt : jt + 1],
                            )
                    # nb3 chain-major (all chunks present by now)
                    for jt in (0, 4):
                        ps3 = pfx.tile([128, 512], F32, tag="pf0_0", name=f"pf3_{jt}")
                        for cch in range(CC):
                            nc.tensor.matmul(
                                ps3[:, :],
                                wt_sb[cch][:, jt * 128 : (jt + 1) * 128],
                                xts[cch][:, 1536:2048],
                                start=(cch == 0),
                                stop=(cch == CC - 1),
                            )
                        nc.scalar.activation(
                            qkT[jt][:, 1536:2048],
                            ps3[:, :],
                            mybir.ActivationFunctionType.Identity,
                            bias=bqk_sb[:, jt : jt + 1],
                        )
                    rope_block(0, 0)
                    rope_block(4, 0)

                # ---- fused attention + fill stream ----
                with (
                    tc.tile_pool(name="attn_ps", bufs=2, space="PSUM") as sp,
                    tc.tile_pool(name="o_ps", bufs=2, space="PSUM") as op,
                    tc.tile_pool(name="fill_ps", bufs=2, space="PSUM") as fp,
                    tc.tile_pool(name="pt_pool", bufs=10) as ptp,
                    tc.tile_pool(name="znorm", bufs=6) as zp,
                    tc.tile_pool(name="onsb_pool", bufs=12) as obp,
                    tc.tile_pool(name="ostage", bufs=8) as osg,
                ):
                    # ---------------- fill queue machinery ----------------
                    v_cache = {}

                    def emit_v_half(t, half):
                        _lbl(f"fill_v{t}")
                        if half == 0:
                            v_cache[t] = fp.tile([128, 512], F32, tag="fill", name=f"psv_{t}")
                        psv = v_cache[t]
                        for cch in range(4 * half, 4 * half + 4):
                            nc.tensor.matmul(
                                psv[:, :],
                                xts[cch][:, t * 128 : (t + 1) * 128],
                                wt_sb[cch][:, 1024:1536],
                                start=(cch == 0),
                                stop=False,
                            )
                        if half == 0:
                            return
                        nc.tensor.matmul(
                            psv[:, :],
                            r(ones_sb[:, 0:128]),
                            r(bv_sb[:, :]),
                            start=False,
                            stop=True,
                        )
                        nc.scalar.copy(
                            vN[t].rearrange("p (h e) -> p h e", e=65)[:, :, 0:64],
                            psv[:, :].rearrange("p (h d) -> p h d", d=64),
                        )

                    def emit_v(t):
                        emit_v_half(t, 0)
                        emit_v_half(t, 1)

                    def emit_qk(jt, nbp):
                        _lbl(f"fill_qk{jt}_{nbp}")
                        ps = fp.tile([128, 512], F32, tag="fill", name=f"psqk_{jt}_{nbp}")
                        for cch in range(CC):
                            nc.tensor.matmul(
                                ps[:, :],
                                wt_sb[cch][:, jt * 128 : (jt + 1) * 128],
                                xts[cch][:, nbp * 512 : (nbp + 1) * 512],
                                start=(cch == 0),
                                stop=(cch == CC - 1),
                            )
                        nc.scalar.activation(
                            qkT[jt][:, nbp * 512 : (nbp + 1) * 512],
                            ps[:, :],
                            mybir.ActivationFunctionType.Identity,
                            bias=bqk_sb[:, jt : jt + 1],
                        )

                    opart = {}
                    op_cache = {}

                    def emit_outproj_i(i, cb, mode="full"):
                        # mode="part": accumulate head-groups 0-2 into SBUF
                        # early; "fin": tail does only the hc3 matmul + add;
                        # "fullA"/"fullB" split a full chain into two items
                        _lbl(f"outproj{i}_{cb}")
                        if mode == "fullA":
                            op_cache[(i, cb)] = fp.tile(
                                [128, 512], F32, tag="fill", name=f"pso_{i}_{cb}"
                            )
                            pso = op_cache[(i, cb)]
                            for hc in range(2):
                                nc.tensor.matmul(
                                    pso[:, :],
                                    oT[hc][:, i * 128 : (i + 1) * 128],
                                    owt_sb[hc][:, cb * 512 : (cb + 1) * 512],
                                    start=(hc == 0),
                                    stop=False,
                                )
                            return
                        if mode == "fullB":
                            pso = op_cache[(i, cb)]
                            for hc in range(2, 4):
                                nc.tensor.matmul(
                                    pso[:, :],
                                    oT[hc][:, i * 128 : (i + 1) * 128],
                                    owt_sb[hc][:, cb * 512 : (cb + 1) * 512],
                                    start=False,
                                    stop=(hc == 3),
                                )
                            ost = osg.tile([128, 512], F32, tag="ost", name=f"ost_{i}_{cb}")
                            nc.vector.tensor_copy(out=ost[:, :], in_=pso[:, :])
                            nc.sync.dma_start(
                                out=out[i * 128 : (i + 1) * 128, cb * 512 : (cb + 1) * 512],
                                in_=ost[:, :],
                            )
                            return
                        pso = fp.tile([128, 512], F32, tag="fill", name=f"pso_{i}_{cb}_{mode}")
                        hcs = {"full": (0, 4), "part": (0, 3), "fin": (3, 4),
                               "pAB": (0, 2), "fCD": (2, 4)}[mode]
                        for hc in range(*hcs):
                            nc.tensor.matmul(
                                pso[:, :],
                                oT[hc][:, i * 128 : (i + 1) * 128],
                                owt_sb[hc][:, cb * 512 : (cb + 1) * 512],
                                start=(hc == hcs[0]),
                                stop=(hc == hcs[1] - 1),
                            )
                        if mode in ("part", "pAB"):
                            pa = pp.tile([128, 512], F32, tag=f"opart{i % 4}_{cb}", name=f"opart_{i}_{cb}")
                            nc.vector.tensor_copy(out=pa[:, :], in_=pso[:, :])
                            opart[(i, cb)] = pa
                            return
                        ost = osg.tile([128, 512], F32, tag="ost", name=f"ost_{i}_{cb}")
                        if mode in ("fin", "fCD"):
                            nc.vector.tensor_add(ost[:, :], pso[:, :], opart[(i, cb)][:, :])
                        else:
                            nc.vector.tensor_copy(out=ost[:, :], in_=pso[:, :])
                        nc.sync.dma_start(
                            out=out[i * 128 : (i + 1) * 128, cb * 512 : (cb + 1) * 512],
                            in_=ost[:, :],
                        )

                    # v tiles 0-3 (needed by the first attention block) and
                    # the remaining pair-0 rope blocks run before attention;
                    # v evacs land early in the DVE queue
                    for t in range(4):
                        emit_v(t)
                    for nbp in range(1, NB):
                        rope_block(0, nbp)
                        rope_block(4, nbp)

                    fill = []  # (level, marker_key_or_None, emitfn)
                    qk_cache = {}

                    def emit_qk_half(jt, nbp, half):
                        # half-chains (4 chunks) share one accumulating pso
                        # bank; finer items track the exp clock more smoothly
                        _lbl(f"fill_qk{jt}_{nbp}")
                        if half == 0:
                            qk_cache[(jt, nbp)] = fp.tile(
                                [128, 512], F32, tag="fill", name=f"psqk_{jt}_{nbp}"
                            )
                        ps = qk_cache[(jt, nbp)]
                        for cch in range(4 * half, 4 * half + 4):
                            nc.tensor.matmul(
                                ps[:, :],
                                wt_sb[cch][:, jt * 128 : (jt + 1) * 128],
                                xts[cch][:, nbp * 512 : (nbp + 1) * 512],
                                start=(cch == 0),
                                stop=(cch == CC - 1),
                            )
                        if half == 1:
                            nc.scalar.activation(
                                qkT[jt][:, nbp * 512 : (nbp + 1) * 512],
                                ps[:, :],
                                mybir.ActivationFunctionType.Identity,
                                bias=bqk_sb[:, jt : jt + 1],
                            )

                    for t in range(4, NT):
                        fill.append((0, None, lambda t=t: emit_v_half(t, 0)))
                        fill.append((0, ("v", t), lambda t=t: emit_v_half(t, 1), "cont"))
                    for p in range(1, 4):
                        # qk chain for one n-block, then its rope right away
                        # (per-block items keep DVE bursts short so the
                        # mask->exp chain is never delayed long); level p-1
                        # paces pair p's chains into head-pair p-1's loop so
                        # late head-pairs keep PE fill against the exp clock
                        for jt in (p, 4 + p):
                            for nbp in range(NB):
                                fill.append(
                                    (p - 1, None, lambda jt=jt, nbp=nbp: emit_qk_half(jt, nbp, 0))
                                )
                                def qk_fin(jt=jt, nbp=nbp):
                                    emit_qk_half(jt, nbp, 1)
                                    rope_block(jt, nbp)
                                fill.append(
                                    (
                                        p - 1,
                                        ("pair", p) if (jt >= 4 and nbp == NB - 1) else None,
                                        qk_fin,
                                        "cont",
                                    )
                                )

                    state = {"pos": 0}
                    done_markers = set()

                    def drain_one(cap):
                        budget = 1.0
                        while state["pos"] < len(fill) and budget > 0:
                            item = fill[state["pos"]]
                            lev, key, fn = item[:3]
                            tag = item[3] if len(item) > 3 else None
                            wt = {"small": 0.4, "med": 0.6}.get(tag, 1.0)
                            if lev > cap or wt > budget + 1e-9:
                                return
                            budget -= wt
                            state["pos"] += 1
                            fn()
                            if key is not None:
                                done_markers.add(key)

                    def drain_until(key):
                        if key in done_markers:
                            return
                        while state["pos"] < len(fill):
                            _lev, k, fn = fill[state["pos"]][:3]
                            state["pos"] += 1
                            fn()
                            if k is not None:
                                done_markers.add(k)
                            if k == key:
                                return

                    def safe_insert_pos(offset):
                        # never split a half-chain pair: continuation items
                        # must stay adjacent to their first half (the shared
                        # PSUM accumulator must not be re-allocated between)
                        p = state["pos"] + offset
                        while p < len(fill) and len(fill[p]) > 3 and fill[p][3] == "cont":
                            p += 1
                        return p

                    def emit_pv(nb, hp, onat, pend, last):
                        pj, ppt, poff2, pw, pooff = pend
                        r0 = pooff // 128
                        for i in range(r0, 4):
                            g, il = i // 2, i % 2
                            # column of q-tile i inside the score window
                            cs = i * 128 - pooff + poff2
                            # bank g's final write happens at the diagonal
                            # j-tile that still covers q-tile g*2+1
                            for hh in range(2):
                                h = 2 * hp + hh
                                nc.tensor.matmul(
                                    onat[g][:, il * 130 + hh * 65 : il * 130 + hh * 65 + 65],
                                    ppt[:, hh * 512 + cs : hh * 512 + cs + 128],
                                    vN[pj][:, h * 65 : h * 65 + 65],
                                    start=(pj == 0 and hh == 0 and il == 0),
                                    stop=(pj == 4 * nb + 2 * g + 1 and hh == 1 and i == g * 2 + 1),
                                    skip_group_check=True,
                                )

                    # ---------------- attention ----------------
                    # pass 1: hp0 then hp1 across all blocks (projection fill
                    # drains here); pass 2 interleaves hp2/hp3 per block so
                    # each block's out-proj becomes fill right after its hp3
                    schedule = [(0, nb) for nb in range(NB)]
                    schedule += [(1, nb) for nb in range(NB)]
                    for nb in range(NB):
                        schedule += [(2, nb), (3, nb)]
                    def flush_pend(pend):
                        # PV for the pending iteration; when it closes a
                        # block, emit that block's normalization too (this
                        # runs AFTER the next block's first scores, so the
                        # exp stream never drains at block boundaries)
                        onat, nb, hp, pj, ppt, poff2, pw, pooff, is_last = pend
                        _lbl(f"pv{hp}_{nb}_{pj}")
                        emit_pv(nb, hp, onat, (pj, ppt, poff2, pw, pooff), last=is_last)
                        if not is_last:
                            return
                        # normalization in natural layout: per-partition 1/Z
                        # broadcast along free dim; the PE transposes that
                        # rebuild oT are deferred as fill items
                        for g in range(2):
                            rzq = zp.tile([128, 4], F32, tag="rz", name=f"rz_{nb}_{hp}_{g}")
                            nc.vector.reciprocal(
                                rzq[:, :], onat[g][:, 64:260:65]
                            )
                            for il in range(2):
                                i = g * 2 + il
                                onsb = obp.tile(
                                    [128, 128], BF16, tag="onsb", name=f"onsb_{nb}_{hp}_{i}"
                                )
                                nc.vector.tensor_mul(
                                    onsb[:, :].rearrange("p (h e) -> p h e", e=64),
                                    onat[g][:, il * 130 : il * 130 + 130].rearrange(
                                        "p (h e) -> p h e", e=65
                                    )[:, :, 0:64],
                                    rzq[:, il * 2 : il * 2 + 2, None].broadcast_to([128, 2, 64]),
                                )

                                def tp_item(nb=nb, hp=hp, i=i, onsb=onsb):
                                    _lbl(f"tp{hp}_{nb}_{i}")
                                    tp = fp.tile([128, 128], BF16, tag="fill", name=f"tp_{nb}_{hp}_{i}")
                                    nc.tensor.transpose(tp[:, :], onsb[:, :], ident_sb[:, :])
                                    nc.vector.tensor_copy(
                                        out=oT[hp][:, nb * 512 + i * 128 : nb * 512 + (i + 1) * 128],
                                        in_=tp[:, :],
                                    )
                                # front of the pending queue: must drain
                                # within the next block so onat/onsb
                                # buffers recycle on time
                                fill.insert(state["pos"] + 2 * g + il, (0, None, tp_item, "small"))
                        if hp == 1 and nb == 3:
                            # block-0 out-proj head-groups 0-1 become fill for
                            # the start of pass 2 (both oT halves are ready)
                            for i in range(4):
                                for cb in range(2):
                                    fill.append(
                                        (0, None, lambda i=i, cb=cb: emit_outproj_i(i, cb, "pAB"), "med")
                                    )
                        if hp == 2 and nb == 3:
                            # last q-block: pre-accumulate head-groups 0-2 so
                            # the tail needs only one matmul + add per chain
                            for i in range(12, 16):
                                for cb in range(2):
                                    fill.append(
                                        (0, None, lambda i=i, cb=cb: emit_outproj_i(i, cb, "part"))
                                    )
                        if hp == 3:
                            # out-proj for q-block nb, one block behind
                            for i in range(4 * nb, 4 * nb + 4):
                                for cb in range(2):
                                    if nb == 3:
                                        fill.append(
                                            (0, None, lambda i=i, cb=cb: emit_outproj_i(i, cb, "fin"), "small")
                                        )
                                    elif nb == 0:
                                        fill.append(
                                            (0, None, lambda i=i, cb=cb: emit_outproj_i(i, cb, "fCD"), "med")
                                        )
                                    else:
                                        fill.append(
                                            (0, None, lambda i=i, cb=cb: emit_outproj_i(i, cb, "fullA"), "med")
                                        )
                                        fill.append(
                                            (0, None, lambda i=i, cb=cb: emit_outproj_i(i, cb, "fullB"), "cont")
                                        )

                    pend = None  # carries the score->exp->PV pipeline
                    # across block boundaries
                    for hp, nb in schedule:
                        if hp > 0:
                            drain_until(("pair", hp))
                        if True:
                            if 4 * nb + 3 >= 4:
                                drain_until(("v", 4 * nb + 3))
                            # natural-layout PV accumulators: one PSUM bank
                            # per 2 q-tiles; col(i%2, h, d) = (i%2)*130+h*65+d
                            # (col 64 of each 65-group is the Z denominator)
                            onat = [
                                op.tile([128, 512], F32, tag="on", name=f"on_{nb}_{hp}_{g}")
                                for g in range(2)
                            ]
                            for j in range(4 * nb + 4):
                                if j // 4 == nb:
                                    qoff = j * 128
                                    w = 512 * (nb + 1) - qoff
                                else:
                                    qoff, w = nb * 512, 512
                                # diag tiles: score/exp only the causal width
                                # w of each head's half; qbase clamp keeps the
                                # window in-bounds at the tail (nb=3), where
                                # the causal range sits at [off2, off2+w)
                                qbase = min(qoff, N - 512)
                                off2 = qoff - qbase
                                ooff = qoff - 512 * nb
                                st = sp.tile([128, 1024], F32, tag="st", name=f"st_{nb}_{hp}_{j}")
                                _lbl(f"score{hp}_{nb}_{j}")
                                dg = j // 4 == nb
                                for hh in range(2):
                                    nc.tensor.matmul(
                                        st[:, hh * 512 + off2 : hh * 512 + off2 + w],
                                        qkT[4 + hp][hh * 64 : hh * 64 + 64, j * 128 : (j + 1) * 128],
                                        qkT[hp][hh * 64 : hh * 64 + 64, qbase + off2 : qbase + off2 + w],
                                        start=True,
                                        stop=True,
                                    )
                                    if dg:
                                        # causal mask on PE: accumulate the
                                        # 0/-1e9 triangle table through an
                                        # identity lhsT (keeps DVE out of the
                                        # exp chain); per-head so each bank's
                                        # start/stop brackets stay sequential
                                        nc.tensor.matmul(
                                            st[:, hh * 512 + off2 : hh * 512 + off2 + 128],
                                            ident_sb[:, :],
                                            mask_sb[:, :],
                                            start=False,
                                            stop=False,
                                            skip_group_check=True,
                                        )
                                pt = ptp.tile([128, 1024], BF16, tag="pt", name=f"pt_{nb}_{hp}_{j}")
                                if w < 512:
                                    nc.scalar.activation(
                                        pt.rearrange("p (b q) -> p b q", b=2)[:, :, off2 : off2 + w],
                                        st[:, 0:1024].rearrange("p (b q) -> p b q", b=2)[:, :, off2 : off2 + w],
                                        mybir.ActivationFunctionType.Exp,
                                    )
                                else:
                                    nc.scalar.activation(
                                        pt[:, :],
                                        st[:, :],
                                        mybir.ActivationFunctionType.Exp,
                                    )
                                # fill BEFORE the PV flush: the fill chain
                                # absorbs the exp wait instead of the PE
                                # head-of-line stalling on it
                                drain_one(0 if hp == 0 else 99)
                                if pend is not None:
                                    flush_pend(pend)
                                pend = (onat, nb, hp, j, pt, off2, w, ooff, j == 4 * nb + 3)
                    flush_pend(pend)
                    while state["pos"] < len(fill):
                        drain_one(99)
    nc.compile()
    return nc


def make_in_maps(x, Wqkv_w, Wqkv_b, out_w):
    """Host-side sharding/layout prep. Returns per-core input dicts."""
    in_maps = []
    # deinterleave perm within one head: even rope components then odd
    perm = np.concatenate([np.arange(0, D, 2), np.arange(1, D, 2)])
    # rope tables
    inv = 1.0 / (ROPE_THETA ** (np.arange(0, D, 2, dtype=np.float64) / D))
    ang = np.arange(N, dtype=np.float64)[:, None] * inv[None, :]  # [N, 32]
    cosT = np.cos(ang).T.astype(np.float32)  # [32, N]
    sinT = np.sin(ang).T.astype(np.float32)
    cosb = np.tile(cosT, (4, 1))  # [128, N]
    sinb = np.concatenate([sinT, -sinT, sinT, -sinT], axis=0)  # [128, N], block a holds out-block a^1's signed sin
    qc, kc = np.arange(128), np.arange(128)
    maskp = np.where(qc[None, :] >= kc[:, None], 0.0, NEG).astype(np.float32)
    identp = np.eye(128, dtype=np.float32)

    for c in range(8):
        b, g = c // 2, c % 2
        heads = np.arange(g * HPC, (g + 1) * HPC)
        qk_rows = (heads[:, None] * D + perm[None, :]).reshape(-1)  # [512]
        v_rows = (heads[:, None] * D + np.arange(D)[None, :]).reshape(-1)
        Wq = Wqkv_w[qk_rows] * SCALE
        bq = Wqkv_b[qk_rows] * SCALE
        Wk = Wqkv_w[C + qk_rows]
        bk = Wqkv_b[C + qk_rows]
        Wv = Wqkv_w[2 * C + v_rows]
        bv = Wqkv_b[2 * C + v_rows]
        Wcat = np.concatenate([Wq, Wk, Wv], axis=0)  # [1536, C]
        wt = np.ascontiguousarray(Wcat.T).reshape(CC, 128, 1536)
        xt = np.ascontiguousarray(x[b].T).reshape(CC, 128, N)
        bqk = np.ascontiguousarray(
            np.concatenate([bq, bk]).reshape(8, 128).T
        )  # [128, 8]
        owt = np.ascontiguousarray(out_w[:, g * JQK : (g + 1) * JQK].T).reshape(
            4, 128, C
        )
        import ml_dtypes
        in_maps.append(
            dict(
                onesp=np.ones((1, 128), dtype=np.float32),
                ones16=np.ones((128, 8), dtype=ml_dtypes.bfloat16),
                xt=xt.astype(ml_dtypes.bfloat16),
                wt=wt.astype(ml_dtypes.bfloat16),
                bqk=bqk.astype(np.float32),
                bv=np.ascontiguousarray(bv[None, :]).astype(np.float32),
                cosb=cosb.astype(ml_dtypes.bfloat16),
                sinb=sinb.astype(ml_dtypes.bfloat16),
                maskb=maskp.astype(ml_dtypes.bfloat16),
                identb=identp.astype(ml_dtypes.bfloat16),
                owt=owt.astype(ml_dtypes.bfloat16),
            )
        )
    return in_maps


_CACHED_NC = None


def kernel(x, Wqkv_w, Wqkv_b, out_w, out_b):
    from concourse.bass_utils import run_bass_kernel_spmd

    global _CACHED_NC
    x = np.asarray(x, dtype=np.float32)
    Wqkv_w = np.asarray(Wqkv_w, dtype=np.float32)
    Wqkv_b = np.asarray(Wqkv_b, dtype=np.float32)
    out_w = np.asarray(out_w, dtype=np.float32)
    out_b = np.asarray(out_b, dtype=np.float32)

    if _CACHED_NC is None:
        _CACHED_NC = build_nc()
    nc = _CACHED_NC
    in_maps = make_in_maps(x, Wqkv_w, Wqkv_b, out_w)
    res = run_bass_kernel_spmd(nc, in_maps, core_ids=list(range(8)))
    out = np.empty((B, N, C), dtype=np.float32)
    for b in range(B):
        out[b] = res.results[2 * b]["out"] + res.results[2 * b + 1]["out"] + out_b
    return out
